# revision 8
# baseline (speedup 1.0000x reference)
"""Trainium2 Bass kernel for MemoryEfficientAttnBlock3D — v3.6.

y = x + conv1x1(attn(conv1x1_{q,k,v}(groupnorm(x))), wp, bp)
x: (2, 64, 32, 32, 8) -> B=2, C=64, N=8192 tokens per batch.
8 cores = 2 batches x 4 query-chunks of 2048 tokens (rotated volumes;
groupnorm stats and softmax reductions are permutation-invariant).

Design (vs the 178us v2 baseline; measured ~133us):
  - 128-partition layouts everywhere: xb/xh are [128, 4096] (token
    halves stacked); kv pair p = (tile p, tile 32+p), so k2[0:64,:] is
    k of the first 4096 tokens and k2[64:128,:] the second -- k/q/v
    projections are clean quadrant matmuls + full-width casts.
  - ACT runs only table-set-0 functions (Exp, Square, Copy, Identity):
    exactly one ACT table load in the whole kernel.
  - exp splits ACT (hw Exp -> fp8, ~1.07us/unit) / DVE (Schraudolph:
    round(a*s+b) as uint8 bits = fp8e4 of e^(s+SHIFT), ~1.31us/unit)
    with a 7:5 weighted round-robin.  GPSIMD cannot read PSUM and is
    ~13x slower elementwise, so it only does memsets + weight DMAs.
  - S scores double-buffer through a 3-deep PSUM pool ([128,1024]
    tiles); AV accumulates per q-block into halves of one [128,1024]
    PSUM tile (DoubleRow fp8, 256-token contraction per pair-column).
    AV matmuls are emitted in batches of 3 so back-to-back same-shape
    matmuls hide the PE's ~173ns SBUF access latency (unit cadence
    ~660ns vs ~790ns unbatched).
  - v^T carries 4 ones columns: the softmax denominator rides the AV
    accumulation in partitions 64-67.  Tails broadcast it via a
    DRAM-bounce DMA mid-kernel (latency fully hidden) or a ones-weights
    fp32 matmul for the final block, then reciprocal_approx_fast.
  - group rstd = Quake bit-hack + 2 fused Newton iterations on DVE
    (no Ln/Sqrt table traffic).
  - input DMA is striped across the sync/scalar/gpsimd queues; stats
    run per-chunk behind the DMA (DVE sums / ACT big-chunk squares).
  - q/k/v production is lazy: only the first 512 tokens of xh plus
    q/k/v for block 0's first pairs are made up front, the rest is
    emitted one step per pair inside block 0.
  - PE warmup matmuls are kept minimal (4+2): they prime the p-state
    ramp but burn HAM (utilization-throttle) credits if overdone.
"""

import numpy as np

import concourse.bass as bass
import concourse.tile as tile
from concourse import bacc, mybir

F32 = mybir.dt.float32
F16 = mybir.dt.float16
F8 = mybir.dt.float8e4
U8 = mybir.dt.uint8
U32 = mybir.dt.uint32
AF = mybir.ActivationFunctionType
OP = mybir.AluOpType
DR = mybir.MatmulPerfMode.DoubleRow

C = 64
C4 = C + 4             # AV out rows: 64 channels + 4 denominator rows
GROUPS = 32
EPS = 1e-6

B_FULL = 2
N_FULL = 8192          # kv tokens per batch
HALF = N_FULL // 2     # 4096
N_CORES = 8
Q_CHUNKS = 4
M_FULL = N_FULL // Q_CHUNKS  # 2048 q tokens per core

MB = 512               # q-token block
NT = 128               # kv tile (tokens); pair p = tiles (p, 32+p)
NPAIR = 32
VSTR = 160             # vt pair stride (fp8); sub A at 0:68, B at 80:148
LAG = 3                # exp units between S and the consuming AV matmul
RING = 3               # S PSUM ring slots

SHIFT = -2.7                   # score shift (softmax-invariant)
A_SCH = 8.0 / np.log(2.0)      # Schraudolph scale for e4m3
DELTA = -0.4                   # rounding-bias tweak
B_SCH = 56.0 + DELTA + A_SCH * SHIFT
RSQRT_MAGIC = 0x5F3759DF

# exp engine weights (measured: ACT ~0.99us/unit, DVE ~1.17us/unit)
W_ACT, W_DVE = 3, 2


def _exp_pattern():
    tot = W_ACT + W_DVE
    pat = []
    acc = {"a": 0.0, "d": 0.0}
    w = {"a": W_ACT, "d": W_DVE}
    for _ in range(tot):
        for k in acc:
            acc[k] += w[k]
        best = max(acc, key=lambda k: acc[k])
        acc[best] -= tot
        pat.append(best)
    return pat


def emit(tc, nc, out_d, xb_d, wq2_d, wk2_d, wpv2_d, bpc_d, pair_d, expand_d,
         dbg=None):
    m_tok = M_FULL
    nch = 8
    sch = HALF // nch  # 512 cols per stats/DMA chunk

    with (
        tc.tile_pool(name="persist", bufs=1) as persist,
        tc.tile_pool(name="exS", bufs=8) as epool,
        tc.tile_pool(name="mtail", bufs=3) as mpool,
        tc.tile_pool(name="spsum", bufs=RING, space="PSUM") as spool,
        tc.tile_pool(name="psump", bufs=1, space="PSUM") as pspool,
        tc.tile_pool(name="dram", bufs=2, space="DRAM") as dpool,
    ):
        av_all = pspool.tile([128, 2 * MB], F32)         # 2 banks: AV halves

        xb_sb = persist.tile([128, HALF], F32)
        xh_sb = persist.tile([128, HALF], F16)
        k2_sb = persist.tile([128, HALF], F16)
        q2_sb = persist.tile([128, m_tok], F16)
        vt_sb = persist.tile([128, NPAIR * VSTR], F8)
        wq2_sb = persist.tile([C, 128], F16)
        wk2_sb = persist.tile([128, C], F16)
        wpv2_sb = persist.tile([128, C], F16)
        bpc_sb = persist.tile([C, 1], F32)
        pair_sb = persist.tile([128, GROUPS], F32)
        expand_sb = persist.tile([GROUPS, 128], F32)
        stats_sb = persist.tile([128, 2 * nch], F32)
        scr_sb = persist.tile([128, sch], F32)
        scr2_sb = persist.tile([128, 2 * sch], F32)
        mrg_sb = persist.tile([GROUPS, 2], F32)
        mrc_sb = persist.tile([128, 2], F32)
        abias_sb = persist.tile([128, 1], F32)
        ones_sb = persist.tile([128, C], F32)
        magic_sb = persist.tile([GROUPS, 1], U32)
        shone_sb = persist.tile([GROUPS, 1], U32)
        wdum_sb = persist.tile([C, C], F16)
        rdum_sb = persist.tile([C, MB], F16)



        # ---- input DMA first, alternating sync/scalar queues; weights
        # follow on the scalar queue (needed only ~15us in)
        for ch in range(nch):
            sl = slice(ch * sch, (ch + 1) * sch)
            eng = (nc.sync, nc.scalar, nc.gpsimd)[ch % 3]
            eng.dma_start(out=xb_sb[:, sl], in_=xb_d[:, sl])
        nc.gpsimd.dma_start(out=wq2_sb[:], in_=wq2_d[:, :])
        nc.gpsimd.dma_start(out=wk2_sb[:], in_=wk2_d[:, :])
        nc.gpsimd.dma_start(out=wpv2_sb[:], in_=wpv2_d[:, :])
        nc.gpsimd.dma_start(out=bpc_sb[:], in_=bpc_d[:, :])
        nc.gpsimd.dma_start(out=pair_sb[:], in_=pair_d[:, :])
        nc.gpsimd.dma_start(out=expand_sb[:], in_=expand_d[:, :])

        nc.gpsimd.memset(abias_sb[:], SHIFT)
        nc.gpsimd.memset(ones_sb[:], 1.0)
        nc.gpsimd.memset(magic_sb[:], RSQRT_MAGIC)
        nc.gpsimd.memset(shone_sb[:], 1)
        nc.gpsimd.memset(wdum_sb[:], 0.0)
        nc.gpsimd.memset(rdum_sb[:], 0.0)
        # ones columns of v^T (fused softmax denominator; 4 copies keep a
        # whole partition group carrying it)
        vt_view = vt_sb[:].rearrange("p (pr s) -> p pr s", s=VSTR)
        nc.gpsimd.memset(vt_view[:, :, C : C + 4], 1.0)
        nc.gpsimd.memset(vt_view[:, :, 80 + C : 80 + C + 4], 1.0)

        # ---- PE warmup during the DMA/stats head (p-state + HAM ramp)
        for i in range(4):
            warm = spool.tile([128, 2 * MB], F32, tag="s", name="warm")
            nc.tensor.matmul(warm[0:C, 0:MB], wdum_sb[:], rdum_sb[:],
                             start=True, stop=True)

        # ---- groupnorm stats at full width: DVE sums, ACT sum-of-squares
        # (Square shares the exp ACT table set -> no extra table load)
        for ch in range(nch):
            sl = slice(ch * sch, (ch + 1) * sch)
            nc.vector.tensor_scalar(
                out=scr_sb[:], in0=xb_sb[:, sl], scalar1=1.0,
                scalar2=None, op0=OP.mult, op1=OP.add,
                accum_out=stats_sb[:, ch : ch + 1],
            )
        sq_spans = [(0, 2), (2, 4), (4, 6), (6, 7), (7, 8)]
        for bc, (c0, c1) in enumerate(sq_spans):
            sl = slice(c0 * sch, c1 * sch)
            nc.scalar.activation(
                out=scr2_sb[:, 0 : (c1 - c0) * sch], in_=xb_sb[:, sl],
                func=AF.Square,
                accum_out=stats_sb[:, nch + bc : nch + bc + 1],
            )
        gpt = spool.tile([128, 2 * MB], F32, tag="s", name="gpt")
        gp = gpt[0:GROUPS, 0 : nch + 5]
        nc.tensor.matmul(gp, pair_sb[:], stats_sb[:, 0 : nch + 5],
                         start=True, stop=True)
        gsum = mpool.tile([GROUPS, 2], F32, tag="gsum")
        nc.vector.tensor_reduce(
            out=gsum[:, 0:1], in_=gp[:, 0:nch],
            axis=mybir.AxisListType.X, op=OP.add,
        )
        nc.vector.tensor_reduce(
            out=gsum[:, 1:2], in_=gp[:, nch : nch + 5],
            axis=mybir.AxisListType.X, op=OP.add,
        )
        # mean = gsum[:,0]; var+eps = gsum[:,1] + eps - mean^2
        msq = mpool.tile([GROUPS, 1], F32, tag="msq")
        nc.vector.tensor_mul(msq[:], gsum[:, 0:1], gsum[:, 0:1])
        ve = mpool.tile([GROUPS, 1], F32, tag="ve")
        nc.vector.scalar_tensor_tensor(
            out=ve[:], in0=gsum[:, 1:2], scalar=EPS, in1=msq[:],
            op0=OP.add, op1=OP.subtract,
        )
        # rstd = rsqrt(ve): Quake bit hack + 2 Newton iterations
        sh = mpool.tile([GROUPS, 1], U32, tag="sh")
        nc.vector.tensor_tensor(
            out=sh[:], in0=ve[:].bitcast(U32), in1=shone_sb[:],
            op=OP.logical_shift_right,
        )
        ya = mpool.tile([GROUPS, 1], F32, tag="ya")
        nc.vector.tensor_tensor(
            out=ya[:].bitcast(U32), in0=magic_sb[:], in1=sh[:],
            op=OP.subtract)
        t1 = mpool.tile([GROUPS, 1], F32, tag="t1n")
        t3 = mpool.tile([GROUPS, 1], F32, tag="t3n")
        yb = mpool.tile([GROUPS, 1], F32, tag="yb")
        for (src, dst) in ((ya, yb), (yb, ya)):
            nc.vector.tensor_mul(t1[:], src[:], src[:])
            nc.vector.tensor_mul(t3[:], ve[:], t1[:])
            nc.vector.tensor_scalar(
                out=t1[:], in0=t3[:], scalar1=-0.5, scalar2=1.5,
                op0=OP.mult, op1=OP.add,
            )
            nc.vector.tensor_mul(dst[:], src[:], t1[:])
        nc.vector.tensor_copy(mrg_sb[:, 0:1], gsum[:, 0:1])
        nc.vector.tensor_copy(mrg_sb[:, 1:2], ya[:])
        ept = spool.tile([128, 2 * MB], F32, tag="s", name="ept")
        ep = ept[:, 0:2]
        nc.tensor.matmul(ep, expand_sb[:], mrg_sb[:], start=True, stop=True)
        nc.vector.tensor_copy(mrc_sb[:], ep)

        # ---- normalize: xh = (x - mean) * rstd, fp16; DVE does
        # (x-mean)*rstd, ACT does Identity(x*rstd + (-mean*rstd))
        nbias = mpool.tile([128, 1], F32, tag="nbias")
        nc.vector.scalar_tensor_tensor(
            out=nbias[:], in0=mrc_sb[:, 0:1], scalar=-1.0,
            in1=mrc_sb[:, 1:2], op0=OP.mult, op1=OP.mult,
        )
        def norm_chunk(i):
            sl = slice(i * MB, (i + 1) * MB)
            if i % 2 == 0:
                nc.vector.tensor_scalar(
                    out=xh_sb[:, sl], in0=xb_sb[:, sl],
                    scalar1=mrc_sb[:, 0:1], scalar2=mrc_sb[:, 1:2],
                    op0=OP.subtract, op1=OP.mult,
                )
            else:
                nc.scalar.activation(
                    out=xh_sb[:, sl], in_=xb_sb[:, sl], func=AF.Identity,
                    scale=mrc_sb[:, 1:2], bias=nbias[:],
                )

        # ---- projections ------------------------------------------------
        # PSUM->SBUF casts alternate DVE tensor_copy / ACT Copy
        cast_rr = [0]

        def cast_copy(dst, src):
            cast_rr[0] += 1
            if cast_rr[0] % 2:
                nc.vector.tensor_copy(dst, src)
            else:
                nc.scalar.activation(out=dst, in_=src, func=AF.Copy)

        def prod_slice():
            t = spool.tile([128, 2 * MB], F32, tag="s", name="prod")
            return t[:, 0:MB]

        # q: wq2 [64,128] duplicates q to both partition halves
        def proj_q(j):
            sl = slice(j * MB, (j + 1) * MB)
            qp = prod_slice()
            nc.tensor.matmul(qp, wq2_sb[:], xh_sb[0:C, sl],
                             start=True, stop=True)
            cast_copy(q2_sb[:, sl], qp)

        # k: chunk c of 512 tokens in each half, concurrent quadrants
        def proj_k(c):
            sl = slice(c * MB, (c + 1) * MB)
            kp = prod_slice()
            nc.tensor.matmul(kp[0:C, :], wk2_sb[0:C, :], xh_sb[0:C, sl],
                             start=True, stop=True)
            nc.tensor.matmul(kp[C:128, :], wk2_sb[C:128, :],
                             xh_sb[C:128, sl], start=True, stop=True)
            cast_copy(k2_sb[:, sl], kp)

        # v^T: transpose via matmul (xh tile as weights); batches write to
        # the idle AV half as scratch, one strided cast into the pair layout
        def proj_v(j, scratch):
            for t in range(4):
                tl = 4 * j + t
                half, tloc = tl // NPAIR, tl % NPAIR
                ro = C * half
                nc.tensor.matmul(
                    scratch[:, t * C : (t + 1) * C],
                    xh_sb[ro : ro + C, tloc * NT : (tloc + 1) * NT],
                    wpv2_sb[ro : ro + C, :],
                    start=True, stop=True,
                )
            base = 4 * j if j < 8 else 4 * (j - 8)
            co = 0 if j < 8 else 80
            cast_copy(
                vt_view[:, base : base + 4, co : co + C],
                scratch[:, 0 : 4 * C].rearrange("p (t m) -> p t m", t=4),
            )

        def proj_v2(cc):
            vsc = spool.tile([128, 2 * MB], F32, tag="s", name="vsc")
            proj_v(cc, vsc[:, 0 : 4 * C])
            proj_v(cc + 8, vsc[:, 4 * C : 8 * C])

        # extra PE warmups right after the stats matmuls: they fill the
        # array while the normalize/production chains catch up
        for i in range(2):
            warm2 = spool.tile([128, 2 * MB], F32, tag="s", name="warm2")
            nc.tensor.matmul(warm2[0:C, 0:MB], wdum_sb[:], rdum_sb[:],
                             start=True, stop=True)

        # minimal pre-sweep production: block 0's first pairs only; the rest
        # is emitted lazily inside block 0 (one step per pair)
        norm_chunk(0)
        proj_q(0)
        proj_k(0)
        proj_v2(0)
        prod_steps = []
        for c in range(1, nch):
            prod_steps.append((norm_chunk, c))
            prod_steps.append((proj_k, c))
            prod_steps.append((proj_v2, c))
        for j in range(1, 4):
            prod_steps.append((proj_q, j))

        # ---- attention: 4 q-blocks, pair-outer within each ---------------
        pat = _exp_pattern()
        pat_n = len(pat)

        def emit_exp(u, sp):
            ex = epool.tile([128, 2 * MB], U8, tag="ex")
            kind = pat[u % pat_n]
            if kind == "a":
                nc.scalar.activation(out=ex[:].bitcast(F8), in_=sp,
                                     func=AF.Exp, bias=abias_sb[:])
            else:
                nc.vector.tensor_scalar(
                    out=ex[:], in0=sp, scalar1=A_SCH, scalar2=B_SCH,
                    op0=OP.mult, op1=OP.add,
                )
            return ex

        def emit_av(p, av, ex):
            nc.tensor.matmul(
                av,
                vt_view[:, p, :].rearrange("p (two m) -> p two m", two=2)[:, :, 0:C4],
                ex[:].bitcast(F8).rearrange("p (two n) -> p two n", two=2),
                start=(p == 0), stop=(p == NPAIR - 1),
                perf_mode=DR,
            )

        # tail: copy AV out, broadcast the denominator row (DRAM-bounce DMA
        # mid-kernel where the latency hides; ones-weights matmul for the
        # final block where the PE is free), reciprocal, multiply, add
        # bias+residual, DMA out
        def make_tail(av, b, last=False):
            msl = slice(b * MB, (b + 1) * MB)
            state = {}

            def stage1(use_act):
                av_sb = mpool.tile([C4, MB], F32, tag="avsb", name="av_sb")
                if use_act:
                    nc.scalar.activation(out=av_sb[:], in_=av, func=AF.Copy)
                else:
                    nc.vector.tensor_copy(av_sb[:], av)
                state["av_sb"] = av_sb

            def stage2():
                av_sb = state["av_sb"]
                if last:
                    dbt = spool.tile([128, 2 * MB], F32, tag="s", name="dbt")
                    den_b = dbt[0:C, 0:MB]
                    nc.tensor.matmul(den_b, ones_sb[C : C + 1, :],
                                     av_sb[C : C + 1, :],
                                     start=True, stop=True)
                else:
                    rd = dpool.tile([1, MB], F32, tag="rd", name="rd")
                    nc.sync.dma_start(out=rd[:], in_=av_sb[C : C + 1, :])
                    den_sb = mpool.tile([C, MB], F32, tag="denb",
                                        name="den_sb")
                    nc.sync.dma_start(out=den_sb[:],
                                      in_=rd[:].to_broadcast([C, MB]))
                    den_b = den_sb[:]
                rec = mpool.tile([C, MB], F32, tag="rec", name="rec")
                nc.vector.reciprocal_approx_fast(out=rec[:], in_=den_b)
                t = mpool.tile([C, MB], F32, tag="tdiv", name="t")
                nc.vector.tensor_mul(t[:], av_sb[0:C, :], rec[:])
                outt = mpool.tile([C, MB], F32, tag="outt", name="outt")
                nc.vector.scalar_tensor_tensor(
                    out=outt[:], in0=t[:], scalar=bpc_sb[:],
                    in1=xb_sb[0:C, msl], op0=OP.add, op1=OP.add,
                )
                nc.sync.dma_start(out=out_d[:, msl], in_=outt[:])

            return stage1, stage2

        tail = None
        unit = 0
        for b in range(4):
            bsl = slice(b * MB, (b + 1) * MB)
            if tail:
                tail[0](b % 2 == 0)
            av = av_all[0:C4, (b % 2) * MB : (b % 2) * MB + MB]
            pending = []
            for p in range(NPAIR):
                if p == 5 and tail:
                    tail[1]()
                    tail = None
                # lazy production: one step per pair during block 0
                if b == 0 and p >= 1 and prod_steps:
                    fn, arg = prod_steps.pop(0)
                    fn(arg)
                sp = spool.tile([128, 2 * MB], F32, tag="s", name="sp")
                nc.tensor.matmul(
                    sp[:, 0:MB], k2_sb[0:C, p * NT : (p + 1) * NT],
                    q2_sb[0:C, bsl], start=True, stop=True,
                )
                nc.tensor.matmul(
                    sp[:, MB : 2 * MB],
                    k2_sb[C:128, p * NT : (p + 1) * NT],
                    q2_sb[C:128, bsl], start=True, stop=True,
                )
                ex = emit_exp(unit, sp[:])
                if dbg is not None and p == 0 and b == 0:
                    scopy = mpool.tile([128, 2 * MB], F32, tag="dbgs")
                    nc.vector.tensor_copy(scopy[:], sp[:])
                    nc.sync.dma_start(out=dbg["sp0"], in_=scopy[:])
                    ecopy = mpool.tile([128, 2 * MB], F32, tag="dbge")
                    nc.vector.tensor_copy(ecopy[:], ex[:])
                    nc.sync.dma_start(out=dbg["ex0"], in_=ecopy[:])
                unit += 1
                pending.append((p, ex))
                # batch AV emission in pairs: back-to-back same-shape
                # matmuls hide more of the PE's SBUF access latency
                if p % 2 == 1 and len(pending) > LAG:
                    n_em = min(2, len(pending) - LAG + 1)
                    for _ in range(n_em):
                        pp, pex = pending.pop(0)
                        emit_av(pp, av, pex)
            for pp, pex in pending:
                emit_av(pp, av, pex)
            if dbg is not None and b == 0:
                avc = mpool.tile([C4, MB], F32, tag="dbgav")
                nc.vector.tensor_copy(avc[:], av)
                nc.sync.dma_start(out=dbg["av0"], in_=avc[:])
            tail = make_tail(av, b, last=(b == 3))

        if dbg is not None:
            nc.sync.dma_start(out=dbg["mrc"], in_=mrc_sb[:])
            nc.sync.dma_start(out=dbg["q2"], in_=q2_sb[:])
            nc.sync.dma_start(out=dbg["k2"], in_=k2_sb[:])
            nc.sync.dma_start(out=dbg["vtb"], in_=vt_sb[:].bitcast(U8))

        # drain the last block's tail
        tail[0](True)
        tail[1]()


def build_program(with_dbg=False):
    nc = bacc.Bacc("TRN2", target_bir_lowering=False, debug=False)
    xb_d = nc.dram_tensor("xb", [128, HALF], F32, kind="ExternalInput")
    wq2_d = nc.dram_tensor("wq2", [C, 128], F16, kind="ExternalInput")
    wk2_d = nc.dram_tensor("wk2", [128, C], F16, kind="ExternalInput")
    wpv2_d = nc.dram_tensor("wpv2", [128, C], F16, kind="ExternalInput")
    bpc_d = nc.dram_tensor("bpc", [C, 1], F32, kind="ExternalInput")
    pair_d = nc.dram_tensor("pair", [128, GROUPS], F32, kind="ExternalInput")
    expand_d = nc.dram_tensor("expand", [GROUPS, 128], F32,
                              kind="ExternalInput")
    out_d = nc.dram_tensor("out", [C, M_FULL], F32, kind="ExternalOutput")
    dbg = None
    if with_dbg:
        dbg = {
            "mrc": nc.dram_tensor("dmrc", [128, 2], F32,
                                  kind="ExternalOutput").ap(),
            "q2": nc.dram_tensor("dq2", [128, M_FULL], F16,
                                 kind="ExternalOutput").ap(),
            "k2": nc.dram_tensor("dk2", [128, HALF], F16,
                                 kind="ExternalOutput").ap(),
            "vtb": nc.dram_tensor("dvtb", [128, NPAIR * VSTR], U8,
                                  kind="ExternalOutput").ap(),
            "sp0": nc.dram_tensor("dsp0", [128, 2 * MB], F32,
                                  kind="ExternalOutput").ap(),
            "ex0": nc.dram_tensor("dex0", [128, 2 * MB], F32,
                                  kind="ExternalOutput").ap(),
            "av0": nc.dram_tensor("dav0", [C4, MB], F32,
                                  kind="ExternalOutput").ap(),
        }
    with tile.TileContext(nc) as tc:
        emit(tc, nc, out_d.ap(), xb_d.ap(), wq2_d.ap(), wk2_d.ap(),
             wpv2_d.ap(), bpc_d.ap(), pair_d.ap(), expand_d.ap(), dbg=dbg)
    nc.compile()
    return nc


def prep_weights(gamma, beta, wq, bq, wk, bk, wv, bv, wp, bp):
    f32 = np.float32
    gamma, beta = gamma.astype(f32), beta.astype(f32)
    scale = f32(1.0) / np.sqrt(f32(C)).astype(f32)
    wq_eff = (wq * gamma[None, :]) * scale
    bq_eff = (wq @ beta + bq) * scale
    wk_eff = wk * gamma[None, :]
    wv_eff = wv * gamma[None, :]
    bv_eff = wv @ beta + bv
    bp_eff = (bp + wp @ bv_eff).astype(f32)
    wpv_eff = (wp @ wv_eff).astype(f32)

    has_c = bool(np.any(bq_eff != 0))

    pair = np.zeros((128, GROUPS), f32)
    idx = np.arange(128)
    pair[idx, (idx % C) // 2] = f32(1.0) / f32(2 * N_FULL)
    expand = np.zeros((GROUPS, 128), f32)
    expand[(idx % C) // 2, idx] = 1.0

    wqT = np.ascontiguousarray(wq_eff.T, f32).astype(np.float16)
    wkT = np.ascontiguousarray(wk_eff.T, f32).astype(np.float16)
    wpvT = np.ascontiguousarray(wpv_eff.T, f32).astype(np.float16)
    shared = {
        "wq2": np.ascontiguousarray(np.concatenate([wqT, wqT], axis=1)),
        "wk2": np.ascontiguousarray(np.concatenate([wkT, wkT], axis=0)),
        "wpv2": np.ascontiguousarray(np.concatenate([wpvT, wpvT], axis=0)),
        "bpc": bp_eff.reshape(C, 1),
        "pair": pair,
        "expand": expand,
    }
    return shared, has_c


_PROGRAM_CACHE = {}


def _get_program():
    if "p" not in _PROGRAM_CACHE:
        _PROGRAM_CACHE["p"] = build_program()
    return _PROGRAM_CACHE["p"]


def make_in_maps(x, shared):
    in_maps = []
    for core in range(N_CORES):
        b, qc = core // Q_CHUNKS, core % Q_CHUNKS
        xb = np.ascontiguousarray(x[b].reshape(C, N_FULL), np.float32)
        xb = np.roll(xb, -qc * M_FULL, axis=1)
        xb128 = np.ascontiguousarray(
            np.concatenate([xb[:, :HALF], xb[:, HALF:]], axis=0))
        in_maps.append({"xb": xb128, **shared})
    return in_maps


def kernel(x, gamma, beta, wq, bq, wk, bk, wv, bv, wp, bp, **run_kwargs):
    from concourse.bass_utils import run_bass_kernel_spmd

    x = np.asarray(x, np.float32)
    shared, has_c = prep_weights(
        np.asarray(gamma), np.asarray(beta), np.asarray(wq), np.asarray(bq),
        np.asarray(wk), np.asarray(bk), np.asarray(wv), np.asarray(bv),
        np.asarray(wp), np.asarray(bp),
    )
    assert not has_c, "v3 kernel assumes zero effective q biases"
    nc = _get_program()
    in_maps = make_in_maps(x, shared)
    res = run_bass_kernel_spmd(nc, in_maps, core_ids=list(range(N_CORES)),
                               **run_kwargs)
    y = np.empty((B_FULL, C, N_FULL), np.float32)
    for core in range(N_CORES):
        b, qc = core // Q_CHUNKS, core % Q_CHUNKS
        y[b, :, qc * M_FULL : (qc + 1) * M_FULL] = res.results[core]["out"]
    out = y.reshape(B_FULL, C, 32, 32, 8)
    if run_kwargs:
        return out, res
    return out


# revision 9
# speedup vs baseline: 1.0105x; 1.0105x over previous
"""Trainium2 Bass kernel for MemoryEfficientAttnBlock3D — v3.6.

y = x + conv1x1(attn(conv1x1_{q,k,v}(groupnorm(x))), wp, bp)
x: (2, 64, 32, 32, 8) -> B=2, C=64, N=8192 tokens per batch.
8 cores = 2 batches x 4 query-chunks of 2048 tokens (rotated volumes;
groupnorm stats and softmax reductions are permutation-invariant).

Design (vs the 178us v2 baseline; measured ~133us):
  - 128-partition layouts everywhere: xb/xh are [128, 4096] (token
    halves stacked); kv pair p = (tile p, tile 32+p), so k2[0:64,:] is
    k of the first 4096 tokens and k2[64:128,:] the second -- k/q/v
    projections are clean quadrant matmuls + full-width casts.
  - ACT runs only table-set-0 functions (Exp, Square, Copy, Identity):
    exactly one ACT table load in the whole kernel.
  - exp splits ACT (hw Exp -> fp8, ~1.07us/unit) / DVE (Schraudolph:
    round(a*s+b) as uint8 bits = fp8e4 of e^(s+SHIFT), ~1.31us/unit)
    with a 7:5 weighted round-robin.  GPSIMD cannot read PSUM and is
    ~13x slower elementwise, so it only does memsets + weight DMAs.
  - S scores double-buffer through a 3-deep PSUM pool ([128,1024]
    tiles); AV accumulates per q-block into halves of one [128,1024]
    PSUM tile (DoubleRow fp8, 256-token contraction per pair-column).
    AV matmuls are emitted in batches of 3 so back-to-back same-shape
    matmuls hide the PE's ~173ns SBUF access latency (unit cadence
    ~660ns vs ~790ns unbatched).
  - v^T carries 4 ones columns: the softmax denominator rides the AV
    accumulation in partitions 64-67.  Tails broadcast it via a
    DRAM-bounce DMA mid-kernel (latency fully hidden) or a ones-weights
    fp32 matmul for the final block, then reciprocal_approx_fast.
  - group rstd = Quake bit-hack + 2 fused Newton iterations on DVE
    (no Ln/Sqrt table traffic).
  - input DMA is striped across the sync/scalar/gpsimd queues; stats
    run per-chunk behind the DMA (DVE sums / ACT big-chunk squares).
  - q/k/v production is lazy: only the first 512 tokens of xh plus
    q/k/v for block 0's first pairs are made up front, the rest is
    emitted one step per pair inside block 0.
  - PE warmup matmuls are kept minimal (4+2): they prime the p-state
    ramp but burn HAM (utilization-throttle) credits if overdone.
"""

import numpy as np

import concourse.bass as bass
import concourse.tile as tile
from concourse import bacc, mybir

F32 = mybir.dt.float32
F16 = mybir.dt.float16
F8 = mybir.dt.float8e4
U8 = mybir.dt.uint8
U32 = mybir.dt.uint32
AF = mybir.ActivationFunctionType
OP = mybir.AluOpType
DR = mybir.MatmulPerfMode.DoubleRow

C = 64
C4 = C + 4             # AV out rows: 64 channels + 4 denominator rows
GROUPS = 32
EPS = 1e-6

B_FULL = 2
N_FULL = 8192          # kv tokens per batch
HALF = N_FULL // 2     # 4096
N_CORES = 8
Q_CHUNKS = 4
M_FULL = N_FULL // Q_CHUNKS  # 2048 q tokens per core

MB = 512               # q-token block
NT = 128               # kv tile (tokens); pair p = tiles (p, 32+p)
NPAIR = 32
VSTR = 160             # vt pair stride (fp8); sub A at 0:68, B at 80:148
LAG = 3                # exp units between S and the consuming AV matmul
RING = 3               # S PSUM ring slots

SHIFT = -2.7                   # score shift (softmax-invariant)
A_SCH = 8.0 / np.log(2.0)      # Schraudolph scale for e4m3
DELTA = -0.4                   # rounding-bias tweak
B_SCH = 56.0 + DELTA + A_SCH * SHIFT
RSQRT_MAGIC = 0x5F3759DF

# exp engine weights (measured: ACT ~0.99us/unit, DVE ~1.17us/unit)
W_ACT, W_DVE = 3, 2


def _exp_pattern():
    tot = W_ACT + W_DVE
    pat = []
    acc = {"a": 0.0, "d": 0.0}
    w = {"a": W_ACT, "d": W_DVE}
    for _ in range(tot):
        for k in acc:
            acc[k] += w[k]
        best = max(acc, key=lambda k: acc[k])
        acc[best] -= tot
        pat.append(best)
    return pat


def emit(tc, nc, out_d, xb_d, wq2_d, wk2_d, wpv2_d, bpc_d, pair_d, expand_d,
         dbg=None):
    m_tok = M_FULL
    nch = 8
    sch = HALF // nch  # 512 cols per stats/DMA chunk

    with (
        tc.tile_pool(name="persist", bufs=1) as persist,
        tc.tile_pool(name="exS", bufs=8) as epool,
        tc.tile_pool(name="mtail", bufs=3) as mpool,
        tc.tile_pool(name="spsum", bufs=RING, space="PSUM") as spool,
        tc.tile_pool(name="psump", bufs=1, space="PSUM") as pspool,
        tc.tile_pool(name="dram", bufs=2, space="DRAM") as dpool,
    ):
        av_all = pspool.tile([128, 2 * MB], F32)         # 2 banks: AV halves

        xb_sb = persist.tile([128, HALF], F32)
        xh_sb = persist.tile([128, HALF], F16)
        k2_sb = persist.tile([128, HALF], F16)
        q2_sb = persist.tile([128, m_tok], F16)
        vt_sb = persist.tile([128, NPAIR * VSTR], F8)
        wq2_sb = persist.tile([C, 128], F16)
        wk2_sb = persist.tile([128, C], F16)
        wpv2_sb = persist.tile([128, C], F16)
        bpc_sb = persist.tile([C, 1], F32)
        pair_sb = persist.tile([128, GROUPS], F32)
        expand_sb = persist.tile([GROUPS, 128], F32)
        stats_sb = persist.tile([128, 2 * nch], F32)
        scr_sb = persist.tile([128, sch], F32)
        scr2_sb = persist.tile([128, 2 * sch], F32)
        mrg_sb = persist.tile([GROUPS, 2], F32)
        mrc_sb = persist.tile([128, 2], F32)
        abias_sb = persist.tile([128, 1], F32)
        ones_sb = persist.tile([128, C], F32)
        magic_sb = persist.tile([GROUPS, 1], U32)
        shone_sb = persist.tile([GROUPS, 1], U32)
        wdum_sb = persist.tile([C, C], F16)
        rdum_sb = persist.tile([C, MB], F16)



        # ---- input DMA first, alternating sync/scalar queues; weights
        # follow on the scalar queue (needed only ~15us in)
        for ch in range(nch):
            sl = slice(ch * sch, (ch + 1) * sch)
            eng = (nc.sync, nc.scalar, nc.gpsimd)[ch % 3]
            eng.dma_start(out=xb_sb[:, sl], in_=xb_d[:, sl])
        nc.gpsimd.dma_start(out=wq2_sb[:], in_=wq2_d[:, :])
        nc.gpsimd.dma_start(out=wk2_sb[:], in_=wk2_d[:, :])
        nc.gpsimd.dma_start(out=wpv2_sb[:], in_=wpv2_d[:, :])
        nc.gpsimd.dma_start(out=bpc_sb[:], in_=bpc_d[:, :])
        nc.gpsimd.dma_start(out=pair_sb[:], in_=pair_d[:, :])
        nc.gpsimd.dma_start(out=expand_sb[:], in_=expand_d[:, :])

        nc.gpsimd.memset(abias_sb[:], SHIFT)
        nc.gpsimd.memset(ones_sb[:], 1.0)
        nc.gpsimd.memset(magic_sb[:], RSQRT_MAGIC)
        nc.gpsimd.memset(shone_sb[:], 1)
        nc.gpsimd.memset(wdum_sb[:], 0.0)
        nc.gpsimd.memset(rdum_sb[:], 0.0)
        # ones columns of v^T (fused softmax denominator; 4 copies keep a
        # whole partition group carrying it)
        vt_view = vt_sb[:].rearrange("p (pr s) -> p pr s", s=VSTR)
        nc.gpsimd.memset(vt_view[:, :, C : C + 4], 1.0)
        nc.gpsimd.memset(vt_view[:, :, 80 + C : 80 + C + 4], 1.0)

        # ---- PE warmup during the DMA/stats head (p-state + HAM ramp)
        for i in range(4):
            warm = spool.tile([128, 2 * MB], F32, tag="s", name="warm")
            nc.tensor.matmul(warm[0:C, 0:MB], wdum_sb[:], rdum_sb[:],
                             start=True, stop=True)

        # ---- groupnorm stats at full width: DVE sums, ACT sum-of-squares
        # (Square shares the exp ACT table set -> no extra table load)
        for ch in range(nch):
            sl = slice(ch * sch, (ch + 1) * sch)
            nc.vector.tensor_scalar(
                out=scr_sb[:], in0=xb_sb[:, sl], scalar1=1.0,
                scalar2=None, op0=OP.mult, op1=OP.add,
                accum_out=stats_sb[:, ch : ch + 1],
            )
        sq_spans = [(0, 2), (2, 4), (4, 6), (6, 7), (7, 8)]
        for bc, (c0, c1) in enumerate(sq_spans):
            sl = slice(c0 * sch, c1 * sch)
            nc.scalar.activation(
                out=scr2_sb[:, 0 : (c1 - c0) * sch], in_=xb_sb[:, sl],
                func=AF.Square,
                accum_out=stats_sb[:, nch + bc : nch + bc + 1],
            )
        gpt = spool.tile([128, 2 * MB], F32, tag="s", name="gpt")
        gp = gpt[0:GROUPS, 0 : nch + 5]
        nc.tensor.matmul(gp, pair_sb[:], stats_sb[:, 0 : nch + 5],
                         start=True, stop=True)
        gsum = mpool.tile([GROUPS, 2], F32, tag="gsum")
        nc.vector.tensor_reduce(
            out=gsum[:, 0:1], in_=gp[:, 0:nch],
            axis=mybir.AxisListType.X, op=OP.add,
        )
        nc.vector.tensor_reduce(
            out=gsum[:, 1:2], in_=gp[:, nch : nch + 5],
            axis=mybir.AxisListType.X, op=OP.add,
        )
        # mean = gsum[:,0]; var+eps = gsum[:,1] + eps - mean^2
        msq = mpool.tile([GROUPS, 1], F32, tag="msq")
        nc.vector.tensor_mul(msq[:], gsum[:, 0:1], gsum[:, 0:1])
        ve = mpool.tile([GROUPS, 1], F32, tag="ve")
        nc.vector.scalar_tensor_tensor(
            out=ve[:], in0=gsum[:, 1:2], scalar=EPS, in1=msq[:],
            op0=OP.add, op1=OP.subtract,
        )
        # rstd = rsqrt(ve): Quake bit hack + 2 Newton iterations
        sh = mpool.tile([GROUPS, 1], U32, tag="sh")
        nc.vector.tensor_tensor(
            out=sh[:], in0=ve[:].bitcast(U32), in1=shone_sb[:],
            op=OP.logical_shift_right,
        )
        ya = mpool.tile([GROUPS, 1], F32, tag="ya")
        nc.vector.tensor_tensor(
            out=ya[:].bitcast(U32), in0=magic_sb[:], in1=sh[:],
            op=OP.subtract)
        t1 = mpool.tile([GROUPS, 1], F32, tag="t1n")
        t3 = mpool.tile([GROUPS, 1], F32, tag="t3n")
        yb = mpool.tile([GROUPS, 1], F32, tag="yb")
        for (src, dst) in ((ya, yb), (yb, ya)):
            nc.vector.tensor_mul(t1[:], src[:], src[:])
            nc.vector.tensor_mul(t3[:], ve[:], t1[:])
            nc.vector.tensor_scalar(
                out=t1[:], in0=t3[:], scalar1=-0.5, scalar2=1.5,
                op0=OP.mult, op1=OP.add,
            )
            nc.vector.tensor_mul(dst[:], src[:], t1[:])
        nc.vector.tensor_copy(mrg_sb[:, 0:1], gsum[:, 0:1])
        nc.vector.tensor_copy(mrg_sb[:, 1:2], ya[:])
        ept = spool.tile([128, 2 * MB], F32, tag="s", name="ept")
        ep = ept[:, 0:2]
        nc.tensor.matmul(ep, expand_sb[:], mrg_sb[:], start=True, stop=True)
        nc.vector.tensor_copy(mrc_sb[:], ep)

        # ---- normalize: xh = (x - mean) * rstd, fp16; DVE does
        # (x-mean)*rstd, ACT does Identity(x*rstd + (-mean*rstd))
        nbias = mpool.tile([128, 1], F32, tag="nbias")
        nc.vector.scalar_tensor_tensor(
            out=nbias[:], in0=mrc_sb[:, 0:1], scalar=-1.0,
            in1=mrc_sb[:, 1:2], op0=OP.mult, op1=OP.mult,
        )
        def norm_chunk(i):
            sl = slice(i * MB, (i + 1) * MB)
            if i % 2 == 0:
                nc.vector.tensor_scalar(
                    out=xh_sb[:, sl], in0=xb_sb[:, sl],
                    scalar1=mrc_sb[:, 0:1], scalar2=mrc_sb[:, 1:2],
                    op0=OP.subtract, op1=OP.mult,
                )
            else:
                nc.scalar.activation(
                    out=xh_sb[:, sl], in_=xb_sb[:, sl], func=AF.Identity,
                    scale=mrc_sb[:, 1:2], bias=nbias[:],
                )

        # ---- projections ------------------------------------------------
        # PSUM->SBUF casts alternate DVE tensor_copy / ACT Copy
        cast_rr = [0]

        def cast_copy(dst, src):
            cast_rr[0] += 1
            if cast_rr[0] % 2:
                nc.vector.tensor_copy(dst, src)
            else:
                nc.scalar.activation(out=dst, in_=src, func=AF.Copy)

        def prod_slice():
            t = spool.tile([128, 2 * MB], F32, tag="s", name="prod")
            return t[:, 0:MB]

        # q: wq2 [64,128] duplicates q to both partition halves
        def proj_q(j):
            sl = slice(j * MB, (j + 1) * MB)
            qp = prod_slice()
            nc.tensor.matmul(qp, wq2_sb[:], xh_sb[0:C, sl],
                             start=True, stop=True)
            cast_copy(q2_sb[:, sl], qp)

        # k: chunk c of 512 tokens in each half, concurrent quadrants
        def proj_k(c):
            sl = slice(c * MB, (c + 1) * MB)
            kp = prod_slice()
            nc.tensor.matmul(kp[0:C, :], wk2_sb[0:C, :], xh_sb[0:C, sl],
                             start=True, stop=True)
            nc.tensor.matmul(kp[C:128, :], wk2_sb[C:128, :],
                             xh_sb[C:128, sl], start=True, stop=True)
            cast_copy(k2_sb[:, sl], kp)

        # v^T: transpose via matmul (xh tile as weights); batches write to
        # the idle AV half as scratch, one strided cast into the pair layout
        def proj_v(j, scratch):
            for t in range(4):
                tl = 4 * j + t
                half, tloc = tl // NPAIR, tl % NPAIR
                ro = C * half
                nc.tensor.matmul(
                    scratch[:, t * C : (t + 1) * C],
                    xh_sb[ro : ro + C, tloc * NT : (tloc + 1) * NT],
                    wpv2_sb[ro : ro + C, :],
                    start=True, stop=True,
                )
            base = 4 * j if j < 8 else 4 * (j - 8)
            co = 0 if j < 8 else 80
            cast_copy(
                vt_view[:, base : base + 4, co : co + C],
                scratch[:, 0 : 4 * C].rearrange("p (t m) -> p t m", t=4),
            )

        def proj_v2(cc):
            vsc = spool.tile([128, 2 * MB], F32, tag="s", name="vsc")
            proj_v(cc, vsc[:, 0 : 4 * C])
            proj_v(cc + 8, vsc[:, 4 * C : 8 * C])

        # extra PE warmups right after the stats matmuls: they fill the
        # array while the normalize/production chains catch up
        for i in range(2):
            warm2 = spool.tile([128, 2 * MB], F32, tag="s", name="warm2")
            nc.tensor.matmul(warm2[0:C, 0:MB], wdum_sb[:], rdum_sb[:],
                             start=True, stop=True)

        # minimal pre-sweep production: block 0's first pairs only; the rest
        # is emitted lazily inside block 0 (one step per pair)
        norm_chunk(0)
        proj_q(0)
        proj_k(0)
        proj_v2(0)
        prod_steps = []
        for c in range(1, nch):
            prod_steps.append((norm_chunk, c))
            prod_steps.append((proj_k, c))
            prod_steps.append((proj_v2, c))
        for j in range(1, 4):
            prod_steps.append((proj_q, j))

        # ---- attention: 4 q-blocks, pair-outer within each ---------------
        pat = _exp_pattern()
        pat_n = len(pat)

        def emit_exp(u, sp):
            ex = epool.tile([128, 2 * MB], U8, tag="ex")
            kind = pat[u % pat_n]
            if kind == "a":
                nc.scalar.activation(out=ex[:].bitcast(F8), in_=sp,
                                     func=AF.Exp, bias=abias_sb[:])
            else:
                nc.vector.tensor_scalar(
                    out=ex[:], in0=sp, scalar1=A_SCH, scalar2=B_SCH,
                    op0=OP.mult, op1=OP.add,
                )
            return ex

        def emit_av(p, av, ex):
            nc.tensor.matmul(
                av,
                vt_view[:, p, :].rearrange("p (two m) -> p two m", two=2)[:, :, 0:C4],
                ex[:].bitcast(F8).rearrange("p (two n) -> p two n", two=2),
                start=(p == 0), stop=(p == NPAIR - 1),
                perf_mode=DR,
            )

        # tail: copy AV out, broadcast the denominator row (DRAM-bounce DMA
        # mid-kernel where the latency hides; ones-weights matmul for the
        # final block where the PE is free), reciprocal, multiply, add
        # bias+residual, DMA out
        def make_tail(av, b, last=False):
            msl = slice(b * MB, (b + 1) * MB)
            state = {}

            def stage1(use_act):
                # split the copy across both engines: each half queues
                # behind a different exp backlog, so the AV bank frees
                # sooner and neither engine pays the full copy
                av_sb = mpool.tile([C4, MB], F32, tag="avsb", name="av_sb")
                nc.vector.tensor_copy(av_sb[0:C4, 0:MB // 2],
                                      av[0:C4, 0:MB // 2])
                nc.scalar.activation(out=av_sb[0:C4, MB // 2 : MB],
                                     in_=av[0:C4, MB // 2 : MB],
                                     func=AF.Copy)
                state["av_sb"] = av_sb

            def stage2():
                av_sb = state["av_sb"]
                if last:
                    dbt = spool.tile([128, 2 * MB], F32, tag="s", name="dbt")
                    den_b = dbt[0:C, 0:MB]
                    nc.tensor.matmul(den_b, ones_sb[C : C + 1, :],
                                     av_sb[C : C + 1, :],
                                     start=True, stop=True)
                else:
                    rd = dpool.tile([1, MB], F32, tag="rd", name="rd")
                    nc.sync.dma_start(out=rd[:], in_=av_sb[C : C + 1, :])
                    den_sb = mpool.tile([C, MB], F32, tag="denb",
                                        name="den_sb")
                    nc.sync.dma_start(out=den_sb[:],
                                      in_=rd[:].to_broadcast([C, MB]))
                    den_b = den_sb[:]
                rec = mpool.tile([C, MB], F32, tag="rec", name="rec")
                nc.vector.reciprocal_approx_fast(out=rec[:], in_=den_b)
                t = mpool.tile([C, MB], F32, tag="tdiv", name="t")
                nc.vector.tensor_mul(t[:], av_sb[0:C, :], rec[:])
                outt = mpool.tile([C, MB], F32, tag="outt", name="outt")
                nc.vector.scalar_tensor_tensor(
                    out=outt[:], in0=t[:], scalar=bpc_sb[:],
                    in1=xb_sb[0:C, msl], op0=OP.add, op1=OP.add,
                )
                nc.sync.dma_start(out=out_d[:, msl], in_=outt[:])

            return stage1, stage2

        tail = None
        unit = 0
        for b in range(4):
            bsl = slice(b * MB, (b + 1) * MB)
            if tail:
                tail[0](b % 2 == 0)
            av = av_all[0:C4, (b % 2) * MB : (b % 2) * MB + MB]
            pending = []
            for p in range(NPAIR):
                if p == 5 and tail:
                    tail[1]()
                    tail = None
                # lazy production: one step per pair during block 0
                if b == 0 and p >= 1 and prod_steps:
                    fn, arg = prod_steps.pop(0)
                    fn(arg)
                sp = spool.tile([128, 2 * MB], F32, tag="s", name="sp")
                nc.tensor.matmul(
                    sp[:, 0:MB], k2_sb[0:C, p * NT : (p + 1) * NT],
                    q2_sb[0:C, bsl], start=True, stop=True,
                )
                nc.tensor.matmul(
                    sp[:, MB : 2 * MB],
                    k2_sb[C:128, p * NT : (p + 1) * NT],
                    q2_sb[C:128, bsl], start=True, stop=True,
                )
                ex = emit_exp(unit, sp[:])
                if dbg is not None and p == 0 and b == 0:
                    scopy = mpool.tile([128, 2 * MB], F32, tag="dbgs")
                    nc.vector.tensor_copy(scopy[:], sp[:])
                    nc.sync.dma_start(out=dbg["sp0"], in_=scopy[:])
                    ecopy = mpool.tile([128, 2 * MB], F32, tag="dbge")
                    nc.vector.tensor_copy(ecopy[:], ex[:])
                    nc.sync.dma_start(out=dbg["ex0"], in_=ecopy[:])
                unit += 1
                pending.append((p, ex))
                # batch AV emission in pairs: back-to-back same-shape
                # matmuls hide more of the PE's SBUF access latency
                if p % 2 == 1 and len(pending) > LAG:
                    n_em = min(2, len(pending) - LAG + 1)
                    for _ in range(n_em):
                        pp, pex = pending.pop(0)
                        emit_av(pp, av, pex)
            for pp, pex in pending:
                emit_av(pp, av, pex)
            if dbg is not None and b == 0:
                avc = mpool.tile([C4, MB], F32, tag="dbgav")
                nc.vector.tensor_copy(avc[:], av)
                nc.sync.dma_start(out=dbg["av0"], in_=avc[:])
            tail = make_tail(av, b, last=(b == 3))

        if dbg is not None:
            nc.sync.dma_start(out=dbg["mrc"], in_=mrc_sb[:])
            nc.sync.dma_start(out=dbg["q2"], in_=q2_sb[:])
            nc.sync.dma_start(out=dbg["k2"], in_=k2_sb[:])
            nc.sync.dma_start(out=dbg["vtb"], in_=vt_sb[:].bitcast(U8))

        # drain the last block's tail
        tail[0](True)
        tail[1]()


def build_program(with_dbg=False):
    nc = bacc.Bacc("TRN2", target_bir_lowering=False, debug=False)
    xb_d = nc.dram_tensor("xb", [128, HALF], F32, kind="ExternalInput")
    wq2_d = nc.dram_tensor("wq2", [C, 128], F16, kind="ExternalInput")
    wk2_d = nc.dram_tensor("wk2", [128, C], F16, kind="ExternalInput")
    wpv2_d = nc.dram_tensor("wpv2", [128, C], F16, kind="ExternalInput")
    bpc_d = nc.dram_tensor("bpc", [C, 1], F32, kind="ExternalInput")
    pair_d = nc.dram_tensor("pair", [128, GROUPS], F32, kind="ExternalInput")
    expand_d = nc.dram_tensor("expand", [GROUPS, 128], F32,
                              kind="ExternalInput")
    out_d = nc.dram_tensor("out", [C, M_FULL], F32, kind="ExternalOutput")
    dbg = None
    if with_dbg:
        dbg = {
            "mrc": nc.dram_tensor("dmrc", [128, 2], F32,
                                  kind="ExternalOutput").ap(),
            "q2": nc.dram_tensor("dq2", [128, M_FULL], F16,
                                 kind="ExternalOutput").ap(),
            "k2": nc.dram_tensor("dk2", [128, HALF], F16,
                                 kind="ExternalOutput").ap(),
            "vtb": nc.dram_tensor("dvtb", [128, NPAIR * VSTR], U8,
                                  kind="ExternalOutput").ap(),
            "sp0": nc.dram_tensor("dsp0", [128, 2 * MB], F32,
                                  kind="ExternalOutput").ap(),
            "ex0": nc.dram_tensor("dex0", [128, 2 * MB], F32,
                                  kind="ExternalOutput").ap(),
            "av0": nc.dram_tensor("dav0", [C4, MB], F32,
                                  kind="ExternalOutput").ap(),
        }
    with tile.TileContext(nc) as tc:
        emit(tc, nc, out_d.ap(), xb_d.ap(), wq2_d.ap(), wk2_d.ap(),
             wpv2_d.ap(), bpc_d.ap(), pair_d.ap(), expand_d.ap(), dbg=dbg)
    nc.compile()
    return nc


def prep_weights(gamma, beta, wq, bq, wk, bk, wv, bv, wp, bp):
    f32 = np.float32
    gamma, beta = gamma.astype(f32), beta.astype(f32)
    scale = f32(1.0) / np.sqrt(f32(C)).astype(f32)
    wq_eff = (wq * gamma[None, :]) * scale
    bq_eff = (wq @ beta + bq) * scale
    wk_eff = wk * gamma[None, :]
    wv_eff = wv * gamma[None, :]
    bv_eff = wv @ beta + bv
    bp_eff = (bp + wp @ bv_eff).astype(f32)
    wpv_eff = (wp @ wv_eff).astype(f32)

    has_c = bool(np.any(bq_eff != 0))

    pair = np.zeros((128, GROUPS), f32)
    idx = np.arange(128)
    pair[idx, (idx % C) // 2] = f32(1.0) / f32(2 * N_FULL)
    expand = np.zeros((GROUPS, 128), f32)
    expand[(idx % C) // 2, idx] = 1.0

    wqT = np.ascontiguousarray(wq_eff.T, f32).astype(np.float16)
    wkT = np.ascontiguousarray(wk_eff.T, f32).astype(np.float16)
    wpvT = np.ascontiguousarray(wpv_eff.T, f32).astype(np.float16)
    shared = {
        "wq2": np.ascontiguousarray(np.concatenate([wqT, wqT], axis=1)),
        "wk2": np.ascontiguousarray(np.concatenate([wkT, wkT], axis=0)),
        "wpv2": np.ascontiguousarray(np.concatenate([wpvT, wpvT], axis=0)),
        "bpc": bp_eff.reshape(C, 1),
        "pair": pair,
        "expand": expand,
    }
    return shared, has_c


_PROGRAM_CACHE = {}


def _get_program():
    if "p" not in _PROGRAM_CACHE:
        _PROGRAM_CACHE["p"] = build_program()
    return _PROGRAM_CACHE["p"]


def make_in_maps(x, shared):
    in_maps = []
    for core in range(N_CORES):
        b, qc = core // Q_CHUNKS, core % Q_CHUNKS
        xb = np.ascontiguousarray(x[b].reshape(C, N_FULL), np.float32)
        xb = np.roll(xb, -qc * M_FULL, axis=1)
        xb128 = np.ascontiguousarray(
            np.concatenate([xb[:, :HALF], xb[:, HALF:]], axis=0))
        in_maps.append({"xb": xb128, **shared})
    return in_maps


def kernel(x, gamma, beta, wq, bq, wk, bk, wv, bv, wp, bp, **run_kwargs):
    from concourse.bass_utils import run_bass_kernel_spmd

    x = np.asarray(x, np.float32)
    shared, has_c = prep_weights(
        np.asarray(gamma), np.asarray(beta), np.asarray(wq), np.asarray(bq),
        np.asarray(wk), np.asarray(bk), np.asarray(wv), np.asarray(bv),
        np.asarray(wp), np.asarray(bp),
    )
    assert not has_c, "v3 kernel assumes zero effective q biases"
    nc = _get_program()
    in_maps = make_in_maps(x, shared)
    res = run_bass_kernel_spmd(nc, in_maps, core_ids=list(range(N_CORES)),
                               **run_kwargs)
    y = np.empty((B_FULL, C, N_FULL), np.float32)
    for core in range(N_CORES):
        b, qc = core // Q_CHUNKS, core % Q_CHUNKS
        y[b, :, qc * M_FULL : (qc + 1) * M_FULL] = res.results[core]["out"]
    out = y.reshape(B_FULL, C, 32, 32, 8)
    if run_kwargs:
        return out, res
    return out


# revision 10
# speedup vs baseline: 1.0147x; 1.0041x over previous
"""Trainium2 Bass kernel for MemoryEfficientAttnBlock3D — v3.6.

y = x + conv1x1(attn(conv1x1_{q,k,v}(groupnorm(x))), wp, bp)
x: (2, 64, 32, 32, 8) -> B=2, C=64, N=8192 tokens per batch.
8 cores = 2 batches x 4 query-chunks of 2048 tokens (rotated volumes;
groupnorm stats and softmax reductions are permutation-invariant).

Design (vs the 178us v2 baseline; measured ~133us):
  - 128-partition layouts everywhere: xb/xh are [128, 4096] (token
    halves stacked); kv pair p = (tile p, tile 32+p), so k2[0:64,:] is
    k of the first 4096 tokens and k2[64:128,:] the second -- k/q/v
    projections are clean quadrant matmuls + full-width casts.
  - ACT runs only table-set-0 functions (Exp, Square, Copy, Identity):
    exactly one ACT table load in the whole kernel.
  - exp splits ACT (hw Exp -> fp8, ~1.07us/unit) / DVE (Schraudolph:
    round(a*s+b) as uint8 bits = fp8e4 of e^(s+SHIFT), ~1.31us/unit)
    with a 7:5 weighted round-robin.  GPSIMD cannot read PSUM and is
    ~13x slower elementwise, so it only does memsets + weight DMAs.
  - S scores double-buffer through a 3-deep PSUM pool ([128,1024]
    tiles); AV accumulates per q-block into halves of one [128,1024]
    PSUM tile (DoubleRow fp8, 256-token contraction per pair-column).
    AV matmuls are emitted in batches of 3 so back-to-back same-shape
    matmuls hide the PE's ~173ns SBUF access latency (unit cadence
    ~660ns vs ~790ns unbatched).
  - v^T carries 4 ones columns: the softmax denominator rides the AV
    accumulation in partitions 64-67.  Tails broadcast it via a
    DRAM-bounce DMA mid-kernel (latency fully hidden) or a ones-weights
    fp32 matmul for the final block, then reciprocal_approx_fast.
  - group rstd = Quake bit-hack + 2 fused Newton iterations on DVE
    (no Ln/Sqrt table traffic).
  - input DMA is striped across the sync/scalar/gpsimd queues; stats
    run per-chunk behind the DMA (DVE sums / ACT big-chunk squares).
  - q/k/v production is lazy: only the first 512 tokens of xh plus
    q/k/v for block 0's first pairs are made up front, the rest is
    emitted one step per pair inside block 0.
  - PE warmup matmuls are kept minimal (4+2): they prime the p-state
    ramp but burn HAM (utilization-throttle) credits if overdone.
"""

import numpy as np

import concourse.bass as bass
import concourse.tile as tile
from concourse import bacc, mybir

F32 = mybir.dt.float32
F16 = mybir.dt.float16
F8 = mybir.dt.float8e4
U8 = mybir.dt.uint8
U32 = mybir.dt.uint32
AF = mybir.ActivationFunctionType
OP = mybir.AluOpType
DR = mybir.MatmulPerfMode.DoubleRow

C = 64
C4 = C + 4             # AV out rows: 64 channels + 4 denominator rows
GROUPS = 32
EPS = 1e-6

B_FULL = 2
N_FULL = 8192          # kv tokens per batch
HALF = N_FULL // 2     # 4096
N_CORES = 8
Q_CHUNKS = 4
M_FULL = N_FULL // Q_CHUNKS  # 2048 q tokens per core

MB = 512               # q-token block
NT = 128               # kv tile (tokens); pair p = tiles (p, 32+p)
NPAIR = 32
VSTR = 160             # vt pair stride (fp8); sub A at 0:68, B at 80:148
LAG = 3                # exp units between S and the consuming AV matmul
RING = 3               # S PSUM ring slots

SHIFT = -2.7                   # score shift (softmax-invariant)
A_SCH = 8.0 / np.log(2.0)      # Schraudolph scale for e4m3
DELTA = -0.4                   # rounding-bias tweak
B_SCH = 56.0 + DELTA + A_SCH * SHIFT
RSQRT_MAGIC = 0x5F3759DF

# exp engine weights (measured: ACT ~0.99us/unit, DVE ~1.17us/unit)
W_ACT, W_DVE = 3, 2


def _exp_pattern():
    tot = W_ACT + W_DVE
    pat = []
    acc = {"a": 0.0, "d": 0.0}
    w = {"a": W_ACT, "d": W_DVE}
    for _ in range(tot):
        for k in acc:
            acc[k] += w[k]
        best = max(acc, key=lambda k: acc[k])
        acc[best] -= tot
        pat.append(best)
    return pat


def emit(tc, nc, out_d, xb_d, wq2_d, wk2_d, wpv2_d, bpc_d, pair_d, expand_d,
         dbg=None):
    m_tok = M_FULL
    nch = 8
    sch = HALF // nch  # 512 cols per stats/DMA chunk

    with (
        tc.tile_pool(name="persist", bufs=1) as persist,
        tc.tile_pool(name="exS", bufs=8) as epool,
        tc.tile_pool(name="mtail", bufs=3) as mpool,
        tc.tile_pool(name="spsum", bufs=RING, space="PSUM") as spool,
        tc.tile_pool(name="psump", bufs=1, space="PSUM") as pspool,
        tc.tile_pool(name="dram", bufs=2, space="DRAM") as dpool,
    ):
        av_all = pspool.tile([128, 2 * MB], F32)         # 2 banks: AV halves

        xb_sb = persist.tile([128, HALF], F16)
        xh_sb = persist.tile([128, HALF], F16)
        k2_sb = persist.tile([128, HALF], F16)
        q2_sb = persist.tile([128, m_tok], F16)
        vt_sb = persist.tile([128, NPAIR * VSTR], F8)
        wq2_sb = persist.tile([C, 128], F16)
        wk2_sb = persist.tile([128, C], F16)
        wpv2_sb = persist.tile([128, C], F16)
        bpc_sb = persist.tile([C, 1], F32)
        pair_sb = persist.tile([128, GROUPS], F32)
        expand_sb = persist.tile([GROUPS, 128], F32)
        stats_sb = persist.tile([128, 2 * nch], F32)
        scr_sb = persist.tile([128, sch], F32)
        scr2_sb = persist.tile([128, 2 * sch], F32)
        mrg_sb = persist.tile([GROUPS, 2], F32)
        mrc_sb = persist.tile([128, 2], F32)
        abias_sb = persist.tile([128, 1], F32)
        ones_sb = persist.tile([128, C], F32)
        magic_sb = persist.tile([GROUPS, 1], U32)
        shone_sb = persist.tile([GROUPS, 1], U32)
        wdum_sb = persist.tile([C, C], F16)
        rdum_sb = persist.tile([C, MB], F16)



        # ---- input DMA first, alternating sync/scalar queues; weights
        # follow on the scalar queue (needed only ~15us in)
        for ch in range(nch):
            sl = slice(ch * sch, (ch + 1) * sch)
            eng = (nc.sync, nc.scalar, nc.gpsimd)[ch % 3]
            eng.dma_start(out=xb_sb[:, sl], in_=xb_d[:, sl])
        nc.gpsimd.dma_start(out=wq2_sb[:], in_=wq2_d[:, :])
        nc.gpsimd.dma_start(out=wk2_sb[:], in_=wk2_d[:, :])
        nc.gpsimd.dma_start(out=wpv2_sb[:], in_=wpv2_d[:, :])
        nc.gpsimd.dma_start(out=bpc_sb[:], in_=bpc_d[:, :])
        nc.gpsimd.dma_start(out=pair_sb[:], in_=pair_d[:, :])
        nc.gpsimd.dma_start(out=expand_sb[:], in_=expand_d[:, :])

        nc.gpsimd.memset(abias_sb[:], SHIFT)
        nc.gpsimd.memset(ones_sb[:], 1.0)
        nc.gpsimd.memset(magic_sb[:], RSQRT_MAGIC)
        nc.gpsimd.memset(shone_sb[:], 1)
        nc.gpsimd.memset(wdum_sb[:], 0.0)
        nc.gpsimd.memset(rdum_sb[:], 0.0)
        # ones columns of v^T (fused softmax denominator; 4 copies keep a
        # whole partition group carrying it)
        vt_view = vt_sb[:].rearrange("p (pr s) -> p pr s", s=VSTR)
        nc.gpsimd.memset(vt_view[:, :, C : C + 4], 1.0)
        nc.gpsimd.memset(vt_view[:, :, 80 + C : 80 + C + 4], 1.0)

        # ---- PE warmup during the DMA/stats head (p-state + HAM ramp)
        for i in range(4):
            warm = spool.tile([128, 2 * MB], F32, tag="s", name="warm")
            nc.tensor.matmul(warm[0:C, 0:MB], wdum_sb[:], rdum_sb[:],
                             start=True, stop=True)

        # ---- groupnorm stats at full width: DVE sums, ACT sum-of-squares
        # (Square shares the exp ACT table set -> no extra table load)
        for ch in range(nch):
            sl = slice(ch * sch, (ch + 1) * sch)
            nc.vector.tensor_scalar(
                out=scr_sb[:], in0=xb_sb[:, sl], scalar1=1.0,
                scalar2=None, op0=OP.mult, op1=OP.add,
                accum_out=stats_sb[:, ch : ch + 1],
            )
        sq_spans = [(0, 2), (2, 4), (4, 6), (6, 7), (7, 8)]
        for bc, (c0, c1) in enumerate(sq_spans):
            sl = slice(c0 * sch, c1 * sch)
            nc.scalar.activation(
                out=scr2_sb[:, 0 : (c1 - c0) * sch], in_=xb_sb[:, sl],
                func=AF.Square,
                accum_out=stats_sb[:, nch + bc : nch + bc + 1],
            )
        gpt = spool.tile([128, 2 * MB], F32, tag="s", name="gpt")
        gp = gpt[0:GROUPS, 0 : nch + 5]
        nc.tensor.matmul(gp, pair_sb[:], stats_sb[:, 0 : nch + 5],
                         start=True, stop=True)
        gsum = mpool.tile([GROUPS, 2], F32, tag="gsum")
        nc.vector.tensor_reduce(
            out=gsum[:, 0:1], in_=gp[:, 0:nch],
            axis=mybir.AxisListType.X, op=OP.add,
        )
        nc.vector.tensor_reduce(
            out=gsum[:, 1:2], in_=gp[:, nch : nch + 5],
            axis=mybir.AxisListType.X, op=OP.add,
        )
        # mean = gsum[:,0]; var+eps = gsum[:,1] + eps - mean^2
        msq = mpool.tile([GROUPS, 1], F32, tag="msq")
        nc.vector.tensor_mul(msq[:], gsum[:, 0:1], gsum[:, 0:1])
        ve = mpool.tile([GROUPS, 1], F32, tag="ve")
        nc.vector.scalar_tensor_tensor(
            out=ve[:], in0=gsum[:, 1:2], scalar=EPS, in1=msq[:],
            op0=OP.add, op1=OP.subtract,
        )
        # rstd = rsqrt(ve): Quake bit hack + 2 Newton iterations
        sh = mpool.tile([GROUPS, 1], U32, tag="sh")
        nc.vector.tensor_tensor(
            out=sh[:], in0=ve[:].bitcast(U32), in1=shone_sb[:],
            op=OP.logical_shift_right,
        )
        ya = mpool.tile([GROUPS, 1], F32, tag="ya")
        nc.vector.tensor_tensor(
            out=ya[:].bitcast(U32), in0=magic_sb[:], in1=sh[:],
            op=OP.subtract)
        t1 = mpool.tile([GROUPS, 1], F32, tag="t1n")
        t3 = mpool.tile([GROUPS, 1], F32, tag="t3n")
        yb = mpool.tile([GROUPS, 1], F32, tag="yb")
        for (src, dst) in ((ya, yb), (yb, ya)):
            nc.vector.tensor_mul(t1[:], src[:], src[:])
            nc.vector.tensor_mul(t3[:], ve[:], t1[:])
            nc.vector.tensor_scalar(
                out=t1[:], in0=t3[:], scalar1=-0.5, scalar2=1.5,
                op0=OP.mult, op1=OP.add,
            )
            nc.vector.tensor_mul(dst[:], src[:], t1[:])
        nc.vector.tensor_copy(mrg_sb[:, 0:1], gsum[:, 0:1])
        nc.vector.tensor_copy(mrg_sb[:, 1:2], ya[:])
        ept = spool.tile([128, 2 * MB], F32, tag="s", name="ept")
        ep = ept[:, 0:2]
        nc.tensor.matmul(ep, expand_sb[:], mrg_sb[:], start=True, stop=True)
        nc.vector.tensor_copy(mrc_sb[:], ep)

        # ---- normalize: xh = (x - mean) * rstd, fp16; DVE does
        # (x-mean)*rstd, ACT does Identity(x*rstd + (-mean*rstd))
        nbias = mpool.tile([128, 1], F32, tag="nbias")
        nc.vector.scalar_tensor_tensor(
            out=nbias[:], in0=mrc_sb[:, 0:1], scalar=-1.0,
            in1=mrc_sb[:, 1:2], op0=OP.mult, op1=OP.mult,
        )
        def norm_chunk(i):
            sl = slice(i * MB, (i + 1) * MB)
            if i % 2 == 0:
                nc.vector.tensor_scalar(
                    out=xh_sb[:, sl], in0=xb_sb[:, sl],
                    scalar1=mrc_sb[:, 0:1], scalar2=mrc_sb[:, 1:2],
                    op0=OP.subtract, op1=OP.mult,
                )
            else:
                nc.scalar.activation(
                    out=xh_sb[:, sl], in_=xb_sb[:, sl], func=AF.Identity,
                    scale=mrc_sb[:, 1:2], bias=nbias[:],
                )

        # ---- projections ------------------------------------------------
        # PSUM->SBUF casts alternate DVE tensor_copy / ACT Copy
        cast_rr = [0]

        def cast_copy(dst, src):
            cast_rr[0] += 1
            if cast_rr[0] % 2:
                nc.vector.tensor_copy(dst, src)
            else:
                nc.scalar.activation(out=dst, in_=src, func=AF.Copy)

        def prod_slice():
            t = spool.tile([128, 2 * MB], F32, tag="s", name="prod")
            return t[:, 0:MB]

        # q: wq2 [64,128] duplicates q to both partition halves
        def proj_q(j):
            sl = slice(j * MB, (j + 1) * MB)
            qp = prod_slice()
            nc.tensor.matmul(qp, wq2_sb[:], xh_sb[0:C, sl],
                             start=True, stop=True)
            cast_copy(q2_sb[:, sl], qp)

        # k: chunk c of 512 tokens in each half, concurrent quadrants
        def proj_k(c):
            sl = slice(c * MB, (c + 1) * MB)
            kp = prod_slice()
            nc.tensor.matmul(kp[0:C, :], wk2_sb[0:C, :], xh_sb[0:C, sl],
                             start=True, stop=True)
            nc.tensor.matmul(kp[C:128, :], wk2_sb[C:128, :],
                             xh_sb[C:128, sl], start=True, stop=True)
            cast_copy(k2_sb[:, sl], kp)

        # v^T: transpose via matmul (xh tile as weights); batches write to
        # the idle AV half as scratch, one strided cast into the pair layout
        def proj_v(j, scratch):
            for t in range(4):
                tl = 4 * j + t
                half, tloc = tl // NPAIR, tl % NPAIR
                ro = C * half
                nc.tensor.matmul(
                    scratch[:, t * C : (t + 1) * C],
                    xh_sb[ro : ro + C, tloc * NT : (tloc + 1) * NT],
                    wpv2_sb[ro : ro + C, :],
                    start=True, stop=True,
                )
            base = 4 * j if j < 8 else 4 * (j - 8)
            co = 0 if j < 8 else 80
            cast_copy(
                vt_view[:, base : base + 4, co : co + C],
                scratch[:, 0 : 4 * C].rearrange("p (t m) -> p t m", t=4),
            )

        def proj_v2(cc):
            vsc = spool.tile([128, 2 * MB], F32, tag="s", name="vsc")
            proj_v(cc, vsc[:, 0 : 4 * C])
            proj_v(cc + 8, vsc[:, 4 * C : 8 * C])

        # extra PE warmups right after the stats matmuls: they fill the
        # array while the normalize/production chains catch up
        for i in range(2):
            warm2 = spool.tile([128, 2 * MB], F32, tag="s", name="warm2")
            nc.tensor.matmul(warm2[0:C, 0:MB], wdum_sb[:], rdum_sb[:],
                             start=True, stop=True)

        # minimal pre-sweep production: block 0's first pairs only; the rest
        # is emitted lazily inside block 0 (one step per pair)
        norm_chunk(0)
        proj_q(0)
        proj_k(0)
        proj_v2(0)
        prod_steps = []
        for c in range(1, nch):
            prod_steps.append((norm_chunk, c))
            prod_steps.append((proj_k, c))
            prod_steps.append((proj_v2, c))
        for j in range(1, 4):
            prod_steps.append((proj_q, j))

        # ---- attention: 4 q-blocks, pair-outer within each ---------------
        pat = _exp_pattern()
        pat_n = len(pat)

        def emit_exp(u, sp):
            ex = epool.tile([128, 2 * MB], U8, tag="ex")
            kind = pat[u % pat_n]
            if kind == "a":
                nc.scalar.activation(out=ex[:].bitcast(F8), in_=sp,
                                     func=AF.Exp, bias=abias_sb[:])
            else:
                nc.vector.tensor_scalar(
                    out=ex[:], in0=sp, scalar1=A_SCH, scalar2=B_SCH,
                    op0=OP.mult, op1=OP.add,
                )
            return ex

        def emit_av(p, av, ex):
            nc.tensor.matmul(
                av,
                vt_view[:, p, :].rearrange("p (two m) -> p two m", two=2)[:, :, 0:C4],
                ex[:].bitcast(F8).rearrange("p (two n) -> p two n", two=2),
                start=(p == 0), stop=(p == NPAIR - 1),
                perf_mode=DR,
            )

        # tail: copy AV out, broadcast the denominator row (DRAM-bounce DMA
        # mid-kernel where the latency hides; ones-weights matmul for the
        # final block where the PE is free), reciprocal, multiply, add
        # bias+residual, DMA out
        def make_tail(av, b, last=False):
            msl = slice(b * MB, (b + 1) * MB)
            state = {}

            def stage1(use_act):
                # split the copy across both engines: each half queues
                # behind a different exp backlog, so the AV bank frees
                # sooner and neither engine pays the full copy
                av_sb = mpool.tile([C4, MB], F32, tag="avsb", name="av_sb")
                nc.vector.tensor_copy(av_sb[0:C4, 0:MB // 2],
                                      av[0:C4, 0:MB // 2])
                nc.scalar.activation(out=av_sb[0:C4, MB // 2 : MB],
                                     in_=av[0:C4, MB // 2 : MB],
                                     func=AF.Copy)
                state["av_sb"] = av_sb

            def stage2():
                av_sb = state["av_sb"]
                if last:
                    dbt = spool.tile([128, 2 * MB], F32, tag="s", name="dbt")
                    den_b = dbt[0:C, 0:MB]
                    nc.tensor.matmul(den_b, ones_sb[C : C + 1, :],
                                     av_sb[C : C + 1, :],
                                     start=True, stop=True)
                else:
                    rd = dpool.tile([1, MB], F32, tag="rd", name="rd")
                    nc.sync.dma_start(out=rd[:], in_=av_sb[C : C + 1, :])
                    den_sb = mpool.tile([C, MB], F32, tag="denb",
                                        name="den_sb")
                    nc.sync.dma_start(out=den_sb[:],
                                      in_=rd[:].to_broadcast([C, MB]))
                    den_b = den_sb[:]
                rec = mpool.tile([C, MB], F32, tag="rec", name="rec")
                nc.vector.reciprocal_approx_fast(out=rec[:], in_=den_b)
                t = mpool.tile([C, MB], F32, tag="tdiv", name="t")
                nc.vector.tensor_mul(t[:], av_sb[0:C, :], rec[:])
                outt = mpool.tile([C, MB], F32, tag="outt", name="outt")
                nc.vector.scalar_tensor_tensor(
                    out=outt[:], in0=t[:], scalar=bpc_sb[:],
                    in1=xb_sb[0:C, msl], op0=OP.add, op1=OP.add,
                )
                nc.sync.dma_start(out=out_d[:, msl], in_=outt[:])

            return stage1, stage2

        tail = None
        unit = 0
        for b in range(4):
            bsl = slice(b * MB, (b + 1) * MB)
            if tail:
                tail[0](b % 2 == 0)
            av = av_all[0:C4, (b % 2) * MB : (b % 2) * MB + MB]
            pending = []
            for p in range(NPAIR):
                if p == 5 and tail:
                    tail[1]()
                    tail = None
                # lazy production: one step per pair during block 0
                if b == 0 and p >= 1 and prod_steps:
                    fn, arg = prod_steps.pop(0)
                    fn(arg)
                sp = spool.tile([128, 2 * MB], F32, tag="s", name="sp")
                nc.tensor.matmul(
                    sp[:, 0:MB], k2_sb[0:C, p * NT : (p + 1) * NT],
                    q2_sb[0:C, bsl], start=True, stop=True,
                )
                nc.tensor.matmul(
                    sp[:, MB : 2 * MB],
                    k2_sb[C:128, p * NT : (p + 1) * NT],
                    q2_sb[C:128, bsl], start=True, stop=True,
                )
                ex = emit_exp(unit, sp[:])
                if dbg is not None and p == 0 and b == 0:
                    scopy = mpool.tile([128, 2 * MB], F32, tag="dbgs")
                    nc.vector.tensor_copy(scopy[:], sp[:])
                    nc.sync.dma_start(out=dbg["sp0"], in_=scopy[:])
                    ecopy = mpool.tile([128, 2 * MB], F32, tag="dbge")
                    nc.vector.tensor_copy(ecopy[:], ex[:])
                    nc.sync.dma_start(out=dbg["ex0"], in_=ecopy[:])
                unit += 1
                pending.append((p, ex))
                # batch AV emission in pairs: back-to-back same-shape
                # matmuls hide more of the PE's SBUF access latency
                if p % 2 == 1 and len(pending) > LAG:
                    n_em = min(2, len(pending) - LAG + 1)
                    for _ in range(n_em):
                        pp, pex = pending.pop(0)
                        emit_av(pp, av, pex)
            for pp, pex in pending:
                emit_av(pp, av, pex)
            if dbg is not None and b == 0:
                avc = mpool.tile([C4, MB], F32, tag="dbgav")
                nc.vector.tensor_copy(avc[:], av)
                nc.sync.dma_start(out=dbg["av0"], in_=avc[:])
            tail = make_tail(av, b, last=(b == 3))

        if dbg is not None:
            nc.sync.dma_start(out=dbg["mrc"], in_=mrc_sb[:])
            nc.sync.dma_start(out=dbg["q2"], in_=q2_sb[:])
            nc.sync.dma_start(out=dbg["k2"], in_=k2_sb[:])
            nc.sync.dma_start(out=dbg["vtb"], in_=vt_sb[:].bitcast(U8))

        # drain the last block's tail
        tail[0](True)
        tail[1]()


def build_program(with_dbg=False):
    nc = bacc.Bacc("TRN2", target_bir_lowering=False, debug=False)
    xb_d = nc.dram_tensor("xb", [128, HALF], F16, kind="ExternalInput")
    wq2_d = nc.dram_tensor("wq2", [C, 128], F16, kind="ExternalInput")
    wk2_d = nc.dram_tensor("wk2", [128, C], F16, kind="ExternalInput")
    wpv2_d = nc.dram_tensor("wpv2", [128, C], F16, kind="ExternalInput")
    bpc_d = nc.dram_tensor("bpc", [C, 1], F32, kind="ExternalInput")
    pair_d = nc.dram_tensor("pair", [128, GROUPS], F32, kind="ExternalInput")
    expand_d = nc.dram_tensor("expand", [GROUPS, 128], F32,
                              kind="ExternalInput")
    out_d = nc.dram_tensor("out", [C, M_FULL], F32, kind="ExternalOutput")
    dbg = None
    if with_dbg:
        dbg = {
            "mrc": nc.dram_tensor("dmrc", [128, 2], F32,
                                  kind="ExternalOutput").ap(),
            "q2": nc.dram_tensor("dq2", [128, M_FULL], F16,
                                 kind="ExternalOutput").ap(),
            "k2": nc.dram_tensor("dk2", [128, HALF], F16,
                                 kind="ExternalOutput").ap(),
            "vtb": nc.dram_tensor("dvtb", [128, NPAIR * VSTR], U8,
                                  kind="ExternalOutput").ap(),
            "sp0": nc.dram_tensor("dsp0", [128, 2 * MB], F32,
                                  kind="ExternalOutput").ap(),
            "ex0": nc.dram_tensor("dex0", [128, 2 * MB], F32,
                                  kind="ExternalOutput").ap(),
            "av0": nc.dram_tensor("dav0", [C4, MB], F32,
                                  kind="ExternalOutput").ap(),
        }
    with tile.TileContext(nc) as tc:
        emit(tc, nc, out_d.ap(), xb_d.ap(), wq2_d.ap(), wk2_d.ap(),
             wpv2_d.ap(), bpc_d.ap(), pair_d.ap(), expand_d.ap(), dbg=dbg)
    nc.compile()
    return nc


def prep_weights(gamma, beta, wq, bq, wk, bk, wv, bv, wp, bp):
    f32 = np.float32
    gamma, beta = gamma.astype(f32), beta.astype(f32)
    scale = f32(1.0) / np.sqrt(f32(C)).astype(f32)
    wq_eff = (wq * gamma[None, :]) * scale
    bq_eff = (wq @ beta + bq) * scale
    wk_eff = wk * gamma[None, :]
    wv_eff = wv * gamma[None, :]
    bv_eff = wv @ beta + bv
    bp_eff = (bp + wp @ bv_eff).astype(f32)
    wpv_eff = (wp @ wv_eff).astype(f32)

    has_c = bool(np.any(bq_eff != 0))

    pair = np.zeros((128, GROUPS), f32)
    idx = np.arange(128)
    pair[idx, (idx % C) // 2] = f32(1.0) / f32(2 * N_FULL)
    expand = np.zeros((GROUPS, 128), f32)
    expand[(idx % C) // 2, idx] = 1.0

    wqT = np.ascontiguousarray(wq_eff.T, f32).astype(np.float16)
    wkT = np.ascontiguousarray(wk_eff.T, f32).astype(np.float16)
    wpvT = np.ascontiguousarray(wpv_eff.T, f32).astype(np.float16)
    shared = {
        "wq2": np.ascontiguousarray(np.concatenate([wqT, wqT], axis=1)),
        "wk2": np.ascontiguousarray(np.concatenate([wkT, wkT], axis=0)),
        "wpv2": np.ascontiguousarray(np.concatenate([wpvT, wpvT], axis=0)),
        "bpc": bp_eff.reshape(C, 1),
        "pair": pair,
        "expand": expand,
    }
    return shared, has_c


_PROGRAM_CACHE = {}


def _get_program():
    if "p" not in _PROGRAM_CACHE:
        _PROGRAM_CACHE["p"] = build_program()
    return _PROGRAM_CACHE["p"]


def make_in_maps(x, shared):
    in_maps = []
    for core in range(N_CORES):
        b, qc = core // Q_CHUNKS, core % Q_CHUNKS
        xb = np.ascontiguousarray(x[b].reshape(C, N_FULL), np.float32)
        xb = np.roll(xb, -qc * M_FULL, axis=1)
        xb128 = np.ascontiguousarray(
            np.concatenate([xb[:, :HALF], xb[:, HALF:]],
                           axis=0).astype(np.float16))
        in_maps.append({"xb": xb128, **shared})
    return in_maps


def kernel(x, gamma, beta, wq, bq, wk, bk, wv, bv, wp, bp, **run_kwargs):
    from concourse.bass_utils import run_bass_kernel_spmd

    x = np.asarray(x, np.float32)
    shared, has_c = prep_weights(
        np.asarray(gamma), np.asarray(beta), np.asarray(wq), np.asarray(bq),
        np.asarray(wk), np.asarray(bk), np.asarray(wv), np.asarray(bv),
        np.asarray(wp), np.asarray(bp),
    )
    assert not has_c, "v3 kernel assumes zero effective q biases"
    nc = _get_program()
    in_maps = make_in_maps(x, shared)
    res = run_bass_kernel_spmd(nc, in_maps, core_ids=list(range(N_CORES)),
                               **run_kwargs)
    y = np.empty((B_FULL, C, N_FULL), np.float32)
    for core in range(N_CORES):
        b, qc = core // Q_CHUNKS, core % Q_CHUNKS
        y[b, :, qc * M_FULL : (qc + 1) * M_FULL] = res.results[core]["out"]
    out = y.reshape(B_FULL, C, 32, 32, 8)
    if run_kwargs:
        return out, res
    return out
